# revision 9
# baseline (speedup 1.0000x reference)
"""CNF forward (vector field + exact Jacobian trace) on 8 TRN2 cores.

Math: reference computes, per sample x (row of state[:, 1:]):
    f(x)  = W3^T tanh(W2^T tanh(W1^T [x; t] + b1) + b2) + b3      (dx)
    trJ   = trace(df/dx)                                          (aug = -trJ)

Closed form of the trace (instead of D=64 JVPs per sample):
    h1 = tanh([x;t] @ W1 + b1),  h2 = tanh(h1 @ W2 + b2)
    s1 = 1 - h1^2
    trJ = sum_h (s1^T F)[b,h] * (1 - h2[b,h]^2)
        = sum_h t2 - sum_h (t2 * h2^2)          (avoids materializing s2)
    with F[h',h] = W2[h',h] * (W3 @ W1[:D])[h, h'] (weights-only, on device)

Sharding: data-parallel, 128 samples per core, weights replicated.

All matmul operands are fp16 (1 cycle/row on the PE vs 4 for fp32's
LOW_HIGH double pass, and half the DMA bytes); accumulation stays in
fp32 PSUM. Values here are O(1) so fp16's range is safe and its 10-bit
mantissa keeps the end-to-end l2 rel err ~5e-4 (gate is 2e-2).

The layer-1 bias b1 + t*W1[D] is folded into the matmul as a 65th
contraction row (ones row in stT) — a per-partition-scalar bias DMA has
16-byte packets and arrives too late otherwise.

Host-side work is layout/cast only (sharding, transposes, fp16 casts,
bias packing); all FLOPs run on device.
"""

import numpy as np

import concourse.bacc as bacc
import concourse.bass as bass
import concourse.tile as tile
from concourse import mybir
from concourse.bass_utils import run_bass_kernel_spmd
from concourse.masks import make_identity
from concourse.tile_rust import add_dep_helper

B, D, H = 1024, 64, 512
NCORES = 8
BC = B // NCORES  # 128 samples per core
KT = H // 128     # 4 feature tiles of 128
F32 = mybir.dt.float32
F16 = mybir.dt.float16
AF = mybir.ActivationFunctionType
ALU = mybir.AluOpType
ts = bass.ts

_NC = {}

# One HWDGE queue per issuing engine; each latency-critical small
# tensor (stT, w1a, w3T) gets the first slot of its own queue. The big
# w2 tiles follow in order of first use; w3cat is needed last.
DMA_PLAN = [
    ("scalar", "stT"), ("sync", "w1a"), ("gpsimd", "w3T"),
    ("scalar", "w2_0"), ("sync", "w2_1"),
    ("gpsimd", "w2_2"), ("gpsimd", "w2_3"),
    ("scalar", "w3cat"),
]


def _build(with_bias23: bool):
    """with_bias23: include rank-1 bias adds for b2/b3 (batch-major layers
    can't take a per-free-dim bias via ACT). setup_inputs() has zero
    biases so the fast path skips them; nonzero b2/b3 still works."""
    nc = bacc.Bacc()

    stT = nc.declare_dram_parameter("stT", [D, BC], F16, isOutput=False)
    W1a = nc.declare_dram_parameter("W1a", [D + 1, H], F16, isOutput=False)
    W2 = nc.declare_dram_parameter("W2", [H, H], F16, isOutput=False)
    # W3 packed as [128, KT*64]: block k holds W3[k*128:(k+1)*128, :]
    W3c = nc.declare_dram_parameter("W3c", [128, KT * D], F16, isOutput=False)
    W3T = nc.declare_dram_parameter("W3T", [D, H], F16, isOutput=False)
    if with_bias23:
        b2r = nc.declare_dram_parameter("b2r", [1, H], F16, isOutput=False)
        b3r = nc.declare_dram_parameter("b3r", [1, D], F16, isOutput=False)
    out = nc.declare_dram_parameter("out", [BC, D + 1], F32, isOutput=True)

    with tile.TileContext(nc) as tc:
        with (
            tc.tile_pool(name="const", bufs=1) as cp,
            tc.tile_pool(name="act", bufs=1) as ap,
            tc.tile_pool(name="ps", bufs=1, space="PSUM") as ps,
        ):
            # ------------- loads (plan set by DMA_PLAN) -------------
            stT_sb = ap.tile([D + 1, BC], F16, tag="stT")
            w1a = cp.tile([D + 1, H], F16, tag="w1a")
            w2_sb = [cp.tile([128, H], F16, tag=f"w2_{k}", name=f"w2_{k}")
                     for k in range(KT)]
            w3T_sb = cp.tile([D, H], F16, tag="w3T")
            w3cat = cp.tile([128, KT * D], F16, tag="w3cat")
            srcs = {"stT": (stT_sb[0:D, :], stT), "w1a": (w1a, W1a),
                    "w3T": (w3T_sb, W3T), "w3cat": (w3cat, W3c)}
            for k in range(KT):
                srcs[f"w2_{k}"] = (w2_sb[k], W2[ts(k, 128), :])
            for eng, nm in DMA_PLAN:
                dst, src = srcs[nm]
                src = src if isinstance(src, bass.AP) else src[:, :]
                getattr(nc, eng).dma_start(out=dst, in_=src)
            # bias rides in contraction row 64: ones row written on-device
            nc.vector.memset(stT_sb[D:D + 1, :], 1.0)
            if with_bias23:
                b2r_sb = cp.tile([1, H], F16, tag="b2r")
                nc.sync.dma_start(out=b2r_sb, in_=b2r[:, :])
                b3r_sb = cp.tile([1, D], F16, tag="b3r")
                nc.sync.dma_start(out=b3r_sb, in_=b3r[:, :])
                onesr = cp.tile([1, BC], F16, tag="onesr")
                nc.vector.memset(onesr, 1.0)
            # fp16 identity for the PE transposes (moving operand dtype
            # sets the transpose rate: fp16 is 1 cycle/row, fp32 is 2)
            ident = cp.tile([128, 128], F16, tag="ident")
            make_identity(nc, ident)

            # ------------- PE warmup -------------
            # HAM clock gate: PE defaults to 1.2 GHz; ~3.4us of sustained
            # matmul activity releases it to 2.4 GHz. Real matmuls can't
            # start until weights land (~9us), so without warmup the
            # whole kernel runs at half clock.
            warm_mv = ap.tile([128, H], F16, tag="warm")
            nc.vector.memset(warm_mv, 0.0)
            warm_last = None
            for _ in range(6):
                wps = ps.tile([128, H], F32, tag="g", bufs=3)
                warm_last = nc.tensor.matmul(wps, warm_mv[:, 0:128],
                                             warm_mv, start=True, stop=True)

            # ------------- layer 1 + trace weights -------------
            # z1 runs first (paced by the tanh pipeline), then the
            # weights-only G = W1x^T @ W3^T matmuls cover the h1 tanh
            # latency; G#3 is deferred into the middle of z2 so z2 can
            # start as soon as h1 is ready (F#3 isn't needed until t2).
            h1, z1_mm = [None] * KT, [None] * KT
            f_sb, g_mm = [None] * KT, [None] * KT

            def emit_z1(j):
                z1_ps = ps.tile([128, BC], F32, tag="z1", bufs=2)
                z1_mm[j] = nc.tensor.matmul(z1_ps, w1a[:, ts(j, 128)],
                                            stT_sb, start=True, stop=True)
                h = ap.tile([128, BC], F16, tag=f"h1_{j}")
                nc.scalar.activation(h, z1_ps, AF.Tanh)
                h1[j] = h

            def emit_g(m):
                g_ps = ps.tile([128, H], F32, tag="g", bufs=3)
                g_mm[m] = nc.tensor.matmul(g_ps, w1a[0:D, ts(m, 128)],
                                           w3T_sb, start=True, stop=True)
                fm = ap.tile([128, H], F16, tag=f"f_{m}")
                nc.vector.tensor_mul(fm, w2_sb[m], g_ps)
                f_sb[m] = fm

            for j in range(KT):
                emit_z1(j)
            add_dep_helper(z1_mm[0].ins, warm_last.ins, sync=False,
                           reason="pe-order z1 after warmup")
            for m in range(KT - 1):
                emit_g(m)
            add_dep_helper(g_mm[0].ins, z1_mm[KT - 1].ins, sync=False,
                           reason="pe-order G after z1")

            # s1 = 1 - h1^2 (gpsimd, feature-major, fp16)
            s1 = []
            for j in range(KT):
                s = ap.tile([128, BC], F16, tag=f"s1_{j}")
                nc.gpsimd.tensor_mul(s, h1[j], h1[j])
                nc.gpsimd.tensor_scalar(s, s, -1.0, 1.0, ALU.mult, ALU.add)
                s1.append(s)

            # ------------- layer 2 (batch-major): h2 -------------
            z2_ps = ps.tile([BC, H], F32, tag="z2", bufs=1)
            z2_mm = []
            for k in range(2):
                z2_mm.append(
                    nc.tensor.matmul(z2_ps, h1[k], w2_sb[k],
                                     start=(k == 0), stop=False))
            add_dep_helper(z2_mm[0].ins, g_mm[KT - 2].ins, sync=False,
                           reason="pe-order z2 after G#2")
            emit_g(KT - 1)
            add_dep_helper(g_mm[KT - 1].ins, z2_mm[1].ins, sync=False,
                           reason="pe-order G#3 inside z2")
            for k in range(2, KT):
                z2_mm.append(
                    nc.tensor.matmul(z2_ps, h1[k], w2_sb[k],
                                     start=False,
                                     stop=(k == KT - 1 and not with_bias23)))
            if with_bias23:
                nc.tensor.matmul(z2_ps, onesr, b2r_sb, start=False, stop=True)
            h2 = ap.tile([BC, H], F16, tag="h2")
            # q = h2^2 - 1 (so aug = -trJ = sum_h t2*q needs no extra terms)
            q2 = ap.tile([BC, H], F16, tag="q2")
            for j in range(KT):
                nc.scalar.activation(h2[:, ts(j, 128)], z2_ps[:, ts(j, 128)],
                                     AF.Tanh)
                eng = nc.gpsimd if j == 1 else nc.vector
                eng.tensor_mul(q2[:, ts(j, 128)], h2[:, ts(j, 128)],
                               h2[:, ts(j, 128)])
                eng.tensor_scalar(q2[:, ts(j, 128)], q2[:, ts(j, 128)],
                                  1.0, -1.0, ALU.mult, ALU.add)

            # ------------- t2 = s1^T F (batch-major) -------------
            t2_ps = ps.tile([BC, H], F32, tag="t2", bufs=1)
            t2_mm = []
            for k in range(KT):
                t2_mm.append(
                    nc.tensor.matmul(t2_ps, s1[k], f_sb[k],
                                     start=(k == 0), stop=(k == KT - 1)))
            add_dep_helper(t2_mm[0].ins, z2_mm[KT - 1].ins, sync=False,
                           reason="pe-order t2 after z2")

            # ------------- layer 3 (batch-major): dx -------------
            # emitted before the aug reduce so the vector-engine hT copies
            # queue ahead of the big multiply/reduce
            final_sb = ap.tile([BC, D + 1], F32, tag="final")
            h2T_sb = []
            for j in range(KT):
                hT_ps = ps.tile([128, BC], F16, tag="z1", bufs=2)
                mm = nc.tensor.transpose(hT_ps, h2[:, ts(j, 128)], ident)
                if j == 0:
                    add_dep_helper(mm.ins, t2_mm[KT - 1].ins, sync=False,
                                   reason="pe-order transpose after t2")
                hT = ap.tile([128, BC], F16, tag=f"h2T_{j}", name=f"hT_{j}")
                if j < 2:
                    nc.vector.tensor_copy(hT, hT_ps)
                else:
                    nc.scalar.copy(hT, hT_ps)
                h2T_sb.append(hT)
            o_ps = ps.tile([BC, D], F32, tag="o", bufs=1)
            for k in range(KT):
                nc.tensor.matmul(o_ps, h2T_sb[k], w3cat[:, ts(k, D)],
                                 start=(k == 0),
                                 stop=(k == KT - 1 and not with_bias23))
            if with_bias23:
                nc.tensor.matmul(o_ps, onesr, b3r_sb, start=False, stop=True)
            nc.scalar.copy(final_sb[:, 1:D + 1], o_ps)

            # aug = -trJ = sum_h t2 * (h2^2 - 1); w_scr in fp16 so the
            # reduce runs at the 16-bit DVE rate
            w_scr = ap.tile([BC, H], F16, tag="w_scr")
            nc.vector.tensor_mul(w_scr, t2_ps, q2)
            nc.vector.tensor_reduce(out=final_sb[:, 0:1], in_=w_scr,
                                    op=ALU.add, axis=mybir.AxisListType.X)
            nc.sync.dma_start(out=out[:, :], in_=final_sb)

    nc.finalize()
    return nc


def _get_nc(with_bias23: bool):
    key = bool(with_bias23)
    if key not in _NC:
        _NC[key] = _build(key)
    return _NC[key]


def make_in_maps(inputs):
    f32 = lambda a: np.ascontiguousarray(np.asarray(a), dtype=np.float32)
    f16 = lambda a: np.ascontiguousarray(np.asarray(a, dtype=np.float32),
                                         dtype=np.float16)
    state = f32(inputs["state"])
    t = float(np.asarray(inputs["t"]).reshape(-1)[0])
    W1 = f32(inputs["W1"])
    b1 = f32(inputs["b1"]).reshape(H)
    W2 = f16(inputs["W2"])
    b2 = f32(inputs["b2"]).reshape(H)
    W3 = f16(inputs["W3"])
    b3 = f32(inputs["b3"]).reshape(D)

    with_bias23 = bool(np.any(b2) or np.any(b3))

    b1_eff = b1 + t * W1[D]                  # fold t-row into bias row
    W1a = np.concatenate([W1[:D], b1_eff[None, :]], axis=0)

    W3c = np.concatenate([W3[k * 128:(k + 1) * 128, :] for k in range(KT)],
                         axis=1)
    base = {
        "W1a": f16(W1a),
        "W2": W2,
        "W3c": np.ascontiguousarray(W3c),
        "W3T": np.ascontiguousarray(W3.T),
    }
    if with_bias23:
        base["b2r"] = f16(b2.reshape(1, H))
        base["b3r"] = f16(b3.reshape(1, D))
    in_maps = []
    for c in range(NCORES):
        m = dict(base)
        m["stT"] = f16(state[c * BC:(c + 1) * BC, 1:].T)
        in_maps.append(m)
    return with_bias23, in_maps


def kernel(**inputs) -> np.ndarray:
    with_bias23, in_maps = make_in_maps(inputs)
    res = run_bass_kernel_spmd(_get_nc(with_bias23), in_maps,
                               list(range(NCORES))).results
    return np.concatenate([res[c]["out"] for c in range(NCORES)], axis=0)



# revision 11
# speedup vs baseline: 1.0146x; 1.0146x over previous
"""CNF forward (vector field + exact Jacobian trace) on 8 TRN2 cores.

Math: reference computes, per sample x (row of state[:, 1:]):
    f(x)  = W3^T tanh(W2^T tanh(W1^T [x; t] + b1) + b2) + b3      (dx)
    trJ   = trace(df/dx)                                          (aug = -trJ)

Closed form of the trace (instead of D=64 JVPs per sample):
    h1 = tanh([x;t] @ W1 + b1),  h2 = tanh(h1 @ W2 + b2)
    s1 = 1 - h1^2
    trJ = sum_h (s1^T F)[b,h] * (1 - h2[b,h]^2)
        = sum_h t2 - sum_h (t2 * h2^2)          (avoids materializing s2)
    with F[h',h] = W2[h',h] * (W3 @ W1[:D])[h, h'] (weights-only, on device)

Sharding: data-parallel, 128 samples per core, weights replicated.

All matmul operands are fp16 (1 cycle/row on the PE vs 4 for fp32's
LOW_HIGH double pass, and half the DMA bytes); accumulation stays in
fp32 PSUM. Values here are O(1) so fp16's range is safe and its 10-bit
mantissa keeps the end-to-end l2 rel err ~5e-4 (gate is 2e-2).

The layer-1 bias b1 + t*W1[D] is folded into the matmul as a 65th
contraction row; the matching ones row of stT is packed on the host.
stT and W1a ride in ONE dram tensor (cmb) so a single descriptor-gen
on the sync queue delivers both z1 operands as early as possible.

The G = W1x^T @ W3^T matmuls have K=64, so pairs of them are packed
into the 128-row PE array via row tiling: W1x and W3^T are host-
duplicated across partitions 0-63 / 64-127 and the two matmuls of a
pair run concurrently in distinct row groups (~2x).

Host-side work is layout/cast only (sharding, transposes, fp16 casts,
bias packing, row duplication); all FLOPs run on device.
"""

import numpy as np

import concourse.bacc as bacc
import concourse.bass as bass
import concourse.tile as tile
from concourse import mybir
from concourse.bass_utils import run_bass_kernel_spmd
from concourse.masks import make_identity
from concourse.tile_rust import add_dep_helper

B, D, H = 1024, 64, 512
NCORES = 8
BC = B // NCORES  # 128 samples per core
KT = H // 128     # 4 feature tiles of 128
F32 = mybir.dt.float32
F16 = mybir.dt.float16
AF = mybir.ActivationFunctionType
ALU = mybir.AluOpType
ts = bass.ts

_NC = {}

# One HWDGE queue per issuing engine; the z1-critical cmb (stT+W1a)
# gets the sync queue to itself. w1d/w3T2 (G inputs) lead gpsimd and
# vector; the w2 tiles follow in order of first use; w3cat is last.
DMA_PLAN = [
    ("sync", "cmb"), ("gpsimd", "w1d"), ("scalar", "w2_0"),
    ("gpsimd", "w3T2"), ("scalar", "w2_1"), ("sync", "w2_3"),
    ("gpsimd", "w2_2"), ("scalar", "w3cat"),
]


def _build(with_bias23: bool):
    """with_bias23: include rank-1 bias adds for b2/b3 (batch-major layers
    can't take a per-free-dim bias via ACT). setup_inputs() has zero
    biases so the fast path skips them; nonzero b2/b3 still works."""
    nc = bacc.Bacc()

    # cmb = [stT with ones row | W1a] : both z1 operands in one DMA
    CMB = nc.declare_dram_parameter("cmb", [D + 1, BC + H], F16, isOutput=False)
    W2 = nc.declare_dram_parameter("W2", [H, H], F16, isOutput=False)
    # W3 packed as [128, KT*64]: block k holds W3[k*128:(k+1)*128, :]
    W3c = nc.declare_dram_parameter("W3c", [128, KT * D], F16, isOutput=False)
    # W1x and W3^T duplicated across the two 64-partition halves for
    # row-packed G pairs
    W1d = nc.declare_dram_parameter("W1d", [128, H], F16, isOutput=False)
    W3T2 = nc.declare_dram_parameter("W3T2", [128, H], F16, isOutput=False)
    if with_bias23:
        b2r = nc.declare_dram_parameter("b2r", [1, H], F16, isOutput=False)
        b3r = nc.declare_dram_parameter("b3r", [1, D], F16, isOutput=False)
    out = nc.declare_dram_parameter("out", [BC, D + 1], F32, isOutput=True)

    with tile.TileContext(nc) as tc:
        with (
            tc.tile_pool(name="const", bufs=1) as cp,
            tc.tile_pool(name="act", bufs=1) as ap,
            tc.tile_pool(name="ps", bufs=1, space="PSUM") as ps,
        ):
            # ------------- loads (plan set by DMA_PLAN) -------------
            cmb_sb = ap.tile([D + 1, BC + H], F16, tag="cmb")
            stT_sb = cmb_sb[:, 0:BC]
            w1a = cmb_sb[:, BC:BC + H]
            w2_sb = [cp.tile([128, H], F16, tag=f"w2_{k}", name=f"w2_{k}")
                     for k in range(KT)]
            w1d_sb = cp.tile([128, H], F16, tag="w1d")
            w3T2_sb = cp.tile([128, H], F16, tag="w3T2")
            w3cat = cp.tile([128, KT * D], F16, tag="w3cat")
            srcs = {"cmb": (cmb_sb, CMB), "w1d": (w1d_sb, W1d),
                    "w3T2": (w3T2_sb, W3T2), "w3cat": (w3cat, W3c)}
            for k in range(KT):
                srcs[f"w2_{k}"] = (w2_sb[k], W2[ts(k, 128), :])
            for eng, nm in DMA_PLAN:
                dst, src = srcs[nm]
                src = src if isinstance(src, bass.AP) else src[:, :]
                dst = dst if isinstance(dst, bass.AP) else dst[:, :]
                getattr(nc, eng).dma_start(out=dst, in_=src)
            if with_bias23:
                b2r_sb = cp.tile([1, H], F16, tag="b2r")
                nc.sync.dma_start(out=b2r_sb, in_=b2r[:, :])
                b3r_sb = cp.tile([1, D], F16, tag="b3r")
                nc.sync.dma_start(out=b3r_sb, in_=b3r[:, :])
                onesr = cp.tile([1, BC], F16, tag="onesr")
                nc.vector.memset(onesr, 1.0)
            # fp16 identity for the PE transposes (moving operand dtype
            # sets the transpose rate: fp16 is 1 cycle/row, fp32 is 2)
            ident = cp.tile([128, 128], F16, tag="ident")
            make_identity(nc, ident)

            # ------------- layer 1 + trace weights -------------
            # z1 runs first (paced by the tanh pipeline); the weights-
            # only G matmuls run as row-packed pairs (K=64 in rows
            # 0-63 / 64-127 concurrently). Pair 1 is deferred into the
            # middle of z2 so z2 starts as soon as h1 is ready.
            h1, z1_mm = [None] * KT, [None] * KT
            f_sb, g_mm = [None] * KT, [None] * KT

            def emit_z1(j):
                z1_ps = ps.tile([128, BC], F32, tag="z1", bufs=2)
                z1_mm[j] = nc.tensor.matmul(z1_ps, w1a[:, ts(j, 128)],
                                            stT_sb, start=True, stop=True)
                h = ap.tile([128, BC], F16, tag=f"h1_{j}")
                nc.scalar.activation(h, z1_ps, AF.Tanh)
                h1[j] = h

            def emit_g(m, lo):
                # lo: partition half (0 or 64) — selects the PE row group
                g_ps = ps.tile([128, H], F32, tag="g", bufs=3)
                g_mm[m] = nc.tensor.matmul(g_ps,
                                           w1d_sb[lo:lo + D, ts(m, 128)],
                                           w3T2_sb[lo:lo + D, :],
                                           start=True, stop=True)
                fm = ap.tile([128, H], F16, tag=f"f_{m}")
                nc.vector.tensor_mul(fm, w2_sb[m], g_ps)
                f_sb[m] = fm

            for j in range(KT):
                emit_z1(j)
            emit_g(0, 0)
            emit_g(1, 64)
            add_dep_helper(g_mm[0].ins, z1_mm[KT - 1].ins, sync=False,
                           reason="pe-order G pair0 after z1")

            # s1 = 1 - h1^2 (gpsimd, feature-major, fp16)
            s1 = []
            for j in range(KT):
                s = ap.tile([128, BC], F16, tag=f"s1_{j}")
                nc.gpsimd.tensor_mul(s, h1[j], h1[j])
                nc.gpsimd.tensor_scalar(s, s, -1.0, 1.0, ALU.mult, ALU.add)
                s1.append(s)

            # ------------- layer 2 (batch-major): h2 -------------
            z2_ps = ps.tile([BC, H], F32, tag="z2", bufs=1)
            z2_mm = []
            for k in range(2):
                z2_mm.append(
                    nc.tensor.matmul(z2_ps, h1[k], w2_sb[k],
                                     start=(k == 0), stop=False))
            add_dep_helper(z2_mm[0].ins, g_mm[1].ins, sync=False,
                           reason="pe-order z2 after G pair0")
            emit_g(2, 0)
            emit_g(3, 64)
            add_dep_helper(g_mm[2].ins, z2_mm[1].ins, sync=False,
                           reason="pe-order G pair1 inside z2")
            for k in range(2, KT):
                z2_mm.append(
                    nc.tensor.matmul(z2_ps, h1[k], w2_sb[k],
                                     start=False,
                                     stop=(k == KT - 1 and not with_bias23)))
            if with_bias23:
                nc.tensor.matmul(z2_ps, onesr, b2r_sb, start=False, stop=True)
            h2 = ap.tile([BC, H], F16, tag="h2")
            # q = h2^2 - 1 (so aug = -trJ = sum_h t2*q needs no extra terms)
            q2 = ap.tile([BC, H], F16, tag="q2")
            for j in range(KT):
                nc.scalar.activation(h2[:, ts(j, 128)], z2_ps[:, ts(j, 128)],
                                     AF.Tanh)
                eng = nc.gpsimd if j == 1 else nc.vector
                eng.tensor_mul(q2[:, ts(j, 128)], h2[:, ts(j, 128)],
                               h2[:, ts(j, 128)])
                eng.tensor_scalar(q2[:, ts(j, 128)], q2[:, ts(j, 128)],
                                  1.0, -1.0, ALU.mult, ALU.add)

            # ------------- t2 = s1^T F (batch-major) -------------
            t2_ps = ps.tile([BC, H], F32, tag="t2", bufs=1)
            t2_mm = []
            for k in range(KT):
                t2_mm.append(
                    nc.tensor.matmul(t2_ps, s1[k], f_sb[k],
                                     start=(k == 0), stop=(k == KT - 1)))
            add_dep_helper(t2_mm[0].ins, z2_mm[KT - 1].ins, sync=False,
                           reason="pe-order t2 after z2")

            # ------------- layer 3 (batch-major): dx -------------
            # emitted before the aug reduce so the vector-engine hT copies
            # queue ahead of the big multiply/reduce
            final_sb = ap.tile([BC, D + 1], F32, tag="final")
            h2T_sb = []
            for j in range(KT):
                hT_ps = ps.tile([128, BC], F16, tag="z1", bufs=2)
                mm = nc.tensor.transpose(hT_ps, h2[:, ts(j, 128)], ident)
                if j == 0:
                    add_dep_helper(mm.ins, t2_mm[KT - 1].ins, sync=False,
                                   reason="pe-order transpose after t2")
                hT = ap.tile([128, BC], F16, tag=f"h2T_{j}", name=f"hT_{j}")
                if j < 2:
                    nc.vector.tensor_copy(hT, hT_ps)
                else:
                    nc.scalar.copy(hT, hT_ps)
                h2T_sb.append(hT)
            o_ps = ps.tile([BC, D], F32, tag="o", bufs=1)
            for k in range(KT):
                nc.tensor.matmul(o_ps, h2T_sb[k], w3cat[:, ts(k, D)],
                                 start=(k == 0),
                                 stop=(k == KT - 1 and not with_bias23))
            if with_bias23:
                nc.tensor.matmul(o_ps, onesr, b3r_sb, start=False, stop=True)
            nc.scalar.copy(final_sb[:, 1:D + 1], o_ps)

            # aug = -trJ = sum_h t2 * (h2^2 - 1); w_scr in fp16 so the
            # reduce runs at the 16-bit DVE rate
            w_scr = ap.tile([BC, H], F16, tag="w_scr")
            nc.vector.tensor_mul(w_scr, t2_ps, q2)
            nc.vector.tensor_reduce(out=final_sb[:, 0:1], in_=w_scr,
                                    op=ALU.add, axis=mybir.AxisListType.X)
            nc.sync.dma_start(out=out[:, :], in_=final_sb)

    nc.finalize()
    return nc


def _get_nc(with_bias23: bool):
    key = bool(with_bias23)
    if key not in _NC:
        _NC[key] = _build(key)
    return _NC[key]


def make_in_maps(inputs):
    f32 = lambda a: np.ascontiguousarray(np.asarray(a), dtype=np.float32)
    f16 = lambda a: np.ascontiguousarray(np.asarray(a, dtype=np.float32),
                                         dtype=np.float16)
    state = f32(inputs["state"])
    t = float(np.asarray(inputs["t"]).reshape(-1)[0])
    W1 = f32(inputs["W1"])
    b1 = f32(inputs["b1"]).reshape(H)
    W2 = f16(inputs["W2"])
    b2 = f32(inputs["b2"]).reshape(H)
    W3 = f16(inputs["W3"])
    b3 = f32(inputs["b3"]).reshape(D)

    with_bias23 = bool(np.any(b2) or np.any(b3))

    b1_eff = b1 + t * W1[D]                  # fold t-row into bias row
    W1a = np.concatenate([W1[:D], b1_eff[None, :]], axis=0)

    W3c = np.concatenate([W3[k * 128:(k + 1) * 128, :] for k in range(KT)],
                         axis=1)
    W1x = W1[:D]
    base = {
        "W2": W2,
        "W3c": np.ascontiguousarray(W3c),
        "W1d": f16(np.concatenate([W1x, W1x], axis=0)),
        "W3T2": f16(np.concatenate([W3.T, W3.T], axis=0)),
    }
    if with_bias23:
        base["b2r"] = f16(b2.reshape(1, H))
        base["b3r"] = f16(b3.reshape(1, D))
    in_maps = []
    for c in range(NCORES):
        m = dict(base)
        stT1 = np.concatenate([state[c * BC:(c + 1) * BC, 1:].T,
                               np.ones((1, BC), np.float32)], axis=0)
        m["cmb"] = f16(np.concatenate([stT1, W1a], axis=1))
        in_maps.append(m)
    return with_bias23, in_maps


def kernel(**inputs) -> np.ndarray:
    with_bias23, in_maps = make_in_maps(inputs)
    res = run_bass_kernel_spmd(_get_nc(with_bias23), in_maps,
                               list(range(NCORES))).results
    return np.concatenate([res[c]["out"] for c in range(NCORES)], axis=0)


# revision 13
# speedup vs baseline: 1.0480x; 1.0330x over previous
"""CNF forward (vector field + exact Jacobian trace) on 8 TRN2 cores.

Math: reference computes, per sample x (row of state[:, 1:]):
    f(x)  = W3^T tanh(W2^T tanh(W1^T [x; t] + b1) + b2) + b3      (dx)
    trJ   = trace(df/dx)                                          (aug = -trJ)

Closed form of the trace (instead of D=64 JVPs per sample):
    h1 = tanh([x;t] @ W1 + b1),  h2 = tanh(h1 @ W2 + b2)
    s1 = 1 - h1^2,  q2 = h2^2 - 1
    aug = -trJ = sum_h (s1^T F)[b,h] * q2[b,h]
    with F[h',h] = W2[h',h] * (W3 @ W1[:D])[h, h'] (weights-only, on device)

Sharding: data-parallel, 128 samples per core, weights replicated.

Everything is FEATURE-MAJOR ([h, b] tiles): z2 and t2 are computed as
16 N=128 matmuls each (same PE cycles as 4 N=512 batch-major ones, the
PE is cold at 1.2 GHz on this part), which makes h2 land feature-major
so dx = h2 @ W3 needs NO PE transposes and no PSUM->SBUF staging
copies. The aug contraction over h (the partition dim) is done on the
PE with a ones[128,1] stationary vector — 4 accumulating matmuls —
instead of a 1.4us serial DVE multiply+reduce.

The k-outer loop order matches DMA arrival: the j-tiles of z2 (resp.
t2) that need W2[k] (resp. F[k]) run as soon as that k-tile lands.

The G = W1x^T @ W3^T matmuls have K=64, so pairs run row-packed in the
128-row PE array concurrently (rows 0-63 / 64-127): W1x is packed
[128, 256] (pair halves) and W3^T is host-duplicated [128, 512].

All matmul operands are fp16 (1 cycle/row on the PE vs 4 for fp32),
accumulation in fp32 PSUM; l2 rel err ~5e-4 (gate 2e-2). The layer-1
bias b1 + t*W1[D] is folded in as a 65th contraction row; the ones row
of stT is packed on the host. PSUM: one 4-bank ring is reused by
z1 -> z2(fm) -> t2(fm) (identical [128,128] f32 shape), 2 banks for the
G pairs, 1 for dx, 1 for aug = exactly 8. Two engines never touch the
same PSUM bank concurrently (hardware PSUM collisions are fatal).

Output is split: out_dx [BC, D] and out_aug [1, BC] (so its DMA rows
are contiguous), concatenated on the host.

Host-side work is layout/cast only (sharding, transposes, fp16 casts,
bias/ones packing, row duplication); all FLOPs run on device.
"""

import numpy as np

import concourse.bacc as bacc
import concourse.bass as bass
import concourse.tile as tile
from concourse import mybir
from concourse.bass_utils import run_bass_kernel_spmd
from concourse.tile_rust import add_dep_helper

B, D, H = 1024, 64, 512
NCORES = 8
BC = B // NCORES  # 128 samples per core
KT = H // 128     # 4 feature tiles of 128
F32 = mybir.dt.float32
F16 = mybir.dt.float16
AF = mybir.ActivationFunctionType
ALU = mybir.AluOpType
ts = bass.ts

_NC = {}

# One HWDGE queue per issuing engine; transfers on a queue run in
# order, ~100-140 GB/s per queue, and nothing moves before ~8us
# (engine wake + descriptor gen), so order matches first use:
# stT/w1a first (z1), then w3T2/w1g (G pair 0), w2 k-tiles, w3cat.
DMA_PLAN = [
    ("scalar", "stT"), ("sync", "w1a"), ("gpsimd", "w3T2"),
    ("scalar", "w2_0"), ("sync", "w2_1"), ("gpsimd", "w1g"),
    ("gpsimd", "w2_2"), ("sync", "w2_3"), ("scalar", "w3cat"),
]


def _build(with_bias23: bool):
    """with_bias23: include rank-1 bias adds for b2/b3. setup_inputs()
    has zero biases so the fast path skips them; nonzero still works."""
    nc = bacc.Bacc()

    # stT rows 0..63 = x^T shard, row 64 = ones (host-packed)
    StT = nc.declare_dram_parameter("stT", [D + 1, BC], F16, isOutput=False)
    W1a = nc.declare_dram_parameter("W1a", [D + 1, H], F16, isOutput=False)
    W2 = nc.declare_dram_parameter("W2", [H, H], F16, isOutput=False)
    # W3 packed as [128, KT*64]: block k holds W3[k*128:(k+1)*128, :]
    W3c = nc.declare_dram_parameter("W3c", [128, KT * D], F16, isOutput=False)
    # G-pair operands: W1x packed [128, 256] (pair halves on partition
    # halves), W3^T duplicated [128, 512]
    W1g = nc.declare_dram_parameter("W1g", [128, 2 * 128], F16, isOutput=False)
    W3T2 = nc.declare_dram_parameter("W3T2", [128, H], F16, isOutput=False)
    if with_bias23:
        b2r = nc.declare_dram_parameter("b2r", [1, H], F16, isOutput=False)
        b3r = nc.declare_dram_parameter("b3r", [1, D], F16, isOutput=False)
    out_dx = nc.declare_dram_parameter("out_dx", [BC, D], F32, isOutput=True)
    out_aug = nc.declare_dram_parameter("out_aug", [1, BC], F32, isOutput=True)

    with tile.TileContext(nc) as tc:
        with (
            tc.tile_pool(name="const", bufs=1) as cp,
            tc.tile_pool(name="act", bufs=1) as ap,
            tc.tile_pool(name="ps", bufs=1, space="PSUM") as ps,
        ):
            # ------------- loads (plan set by DMA_PLAN) -------------
            stT_sb = ap.tile([D + 1, BC], F16, tag="stT")
            w1a = cp.tile([D + 1, H], F16, tag="w1a")
            w2_sb = [cp.tile([128, H], F16, tag=f"w2_{k}", name=f"w2_{k}")
                     for k in range(KT)]
            w1g_sb = cp.tile([128, 2 * 128], F16, tag="w1g")
            w3T2_sb = cp.tile([128, H], F16, tag="w3T2")
            w3cat = cp.tile([128, KT * D], F16, tag="w3cat")
            srcs = {"stT": (stT_sb, StT), "w1a": (w1a, W1a),
                    "w1g": (w1g_sb, W1g), "w3T2": (w3T2_sb, W3T2),
                    "w3cat": (w3cat, W3c)}
            for k in range(KT):
                srcs[f"w2_{k}"] = (w2_sb[k], W2[ts(k, 128), :])
            for eng, nm in DMA_PLAN:
                dst, src = srcs[nm]
                src = src if isinstance(src, bass.AP) else src[:, :]
                dst = dst if isinstance(dst, bass.AP) else dst[:, :]
                getattr(nc, eng).dma_start(out=dst, in_=src)
            if with_bias23:
                b2r_sb = cp.tile([1, H], F16, tag="b2r")
                nc.sync.dma_start(out=b2r_sb, in_=b2r[:, :])
                b3r_sb = cp.tile([1, D], F16, tag="b3r")
                nc.sync.dma_start(out=b3r_sb, in_=b3r[:, :])
                onesr = cp.tile([1, BC], F16, tag="onesr")
                nc.vector.memset(onesr, 1.0)
            # ones column: stationary vector for the aug reduction
            onesc = cp.tile([128, 1], F16, tag="onesc")
            nc.vector.memset(onesc, 1.0)

            # ------------- layer 1 + trace weights -------------
            h1, z1_mm = [None] * KT, [None] * KT
            f_sb, g_mm = [None] * KT, [None] * KT

            def emit_z1(j):
                z1_ps = ps.tile([128, BC], F32, tag="fm", bufs=4)
                z1_mm[j] = nc.tensor.matmul(z1_ps, w1a[:, ts(j, 128)],
                                            stT_sb, start=True, stop=True)
                h = ap.tile([128, BC], F16, tag=f"h1_{j}")
                nc.scalar.activation(h, z1_ps, AF.Tanh)
                h1[j] = h

            def emit_g(m):
                # pair p = m//2 in the two 64-partition row groups
                p, lo = m // 2, 64 * (m % 2)
                g_ps = ps.tile([128, H], F32, tag="g", bufs=2)
                g_mm[m] = nc.tensor.matmul(g_ps,
                                           w1g_sb[lo:lo + D, ts(p, 128)],
                                           w3T2_sb[lo:lo + D, :],
                                           start=True, stop=True)
                fm = ap.tile([128, H], F16, tag=f"f_{m}")
                nc.vector.tensor_mul(fm, w2_sb[m], g_ps)
                f_sb[m] = fm

            for j in range(KT):
                emit_z1(j)
            emit_g(0)
            emit_g(1)
            add_dep_helper(g_mm[0].ins, z1_mm[KT - 1].ins, sync=False,
                           reason="pe-order G pair0 after z1")

            # s1 = 1 - h1^2 (gpsimd, feature-major, fp16)
            s1 = []
            for j in range(KT):
                s = ap.tile([128, BC], F16, tag=f"s1_{j}")
                nc.gpsimd.tensor_mul(s, h1[j], h1[j])
                nc.gpsimd.tensor_scalar(s, s, -1.0, 1.0, ALU.mult, ALU.add)
                s1.append(s)

            # ------------- layer 2 (feature-major): h2 -------------
            # k-outer: all four j-tiles accumulate W2[k]-contributions as
            # soon as w2_k lands. 4 PSUM banks (the fm ring) live at once.
            z2_ps = [ps.tile([128, BC], F32, tag="fm", bufs=4,
                             name=f"z2_{j}") for j in range(KT)]
            z2_mm = [[None] * KT for _ in range(KT)]  # [k][j]
            for k in range(KT):
                for j in range(KT):
                    z2_mm[k][j] = nc.tensor.matmul(
                        z2_ps[j], w2_sb[k][:, ts(j, 128)], h1[k],
                        start=(k == 0), stop=(k == KT - 1 and not with_bias23))
            add_dep_helper(z2_mm[0][0].ins, g_mm[1].ins, sync=False,
                           reason="pe-order z2 after G pair0")
            emit_g(2)
            emit_g(3)
            add_dep_helper(g_mm[2].ins, z2_mm[0][KT - 1].ins, sync=False,
                           reason="pe-order G pair1 inside z2")
            add_dep_helper(z2_mm[1][0].ins, g_mm[3].ins, sync=False,
                           reason="pe-order z2 k1 after G pair1")
            if with_bias23:
                for j in range(KT):
                    nc.tensor.matmul(z2_ps[j], b2r_sb[:, ts(j, 128)], onesr,
                                     start=False, stop=True)
            h2, q2 = [None] * KT, [None] * KT
            for j in range(KT):
                h = ap.tile([128, BC], F16, tag=f"h2_{j}")
                nc.scalar.activation(h, z2_ps[j], AF.Tanh)
                h2[j] = h
                # q = h2^2 - 1 (aug = sum t2*q needs no extra terms)
                q = ap.tile([128, BC], F16, tag=f"q2_{j}")
                eng = nc.gpsimd if j % 2 else nc.vector
                eng.tensor_mul(q, h, h)
                eng.tensor_scalar(q, q, 1.0, -1.0, ALU.mult, ALU.add)
                q2[j] = q

            # ------------- t2 = F^T s1 (feature-major) -------------
            t2_ps = [ps.tile([128, BC], F32, tag="fm", bufs=4,
                             name=f"t2_{j}") for j in range(KT)]
            t2_mm = [[None] * KT for _ in range(KT)]
            for k in range(KT):
                for j in range(KT):
                    t2_mm[k][j] = nc.tensor.matmul(
                        t2_ps[j], f_sb[k][:, ts(j, 128)], s1[k],
                        start=(k == 0), stop=(k == KT - 1))
            add_dep_helper(t2_mm[0][0].ins, z2_mm[KT - 1][KT - 1].ins,
                           sync=False, reason="pe-order t2 after z2")

            # ------------- layer 3: dx = h2 @ W3 (batch-major out) ----
            o_ps = ps.tile([BC, D], F32, tag="o", bufs=1)
            o_mm = [None] * KT
            for j in range(KT):
                o_mm[j] = nc.tensor.matmul(o_ps, h2[j], w3cat[:, ts(j, D)],
                                           start=(j == 0),
                                           stop=(j == KT - 1
                                                 and not with_bias23))
            if with_bias23:
                nc.tensor.matmul(o_ps, onesr, b3r_sb, start=False, stop=True)
            add_dep_helper(o_mm[0].ins, t2_mm[KT - 1][KT - 1].ins,
                           sync=False, reason="pe-order dx after t2")
            final_dx = ap.tile([BC, D], F32, tag="final_dx")
            nc.scalar.copy(final_dx, o_ps)
            nc.scalar.dma_start(out=out_dx[:, :], in_=final_dx)

            # ------------- aug = sum_h t2*q2 via PE ones-reduction ----
            aug_ps = ps.tile([1, BC], F32, tag="aug", bufs=1)
            for j in range(KT):
                w = ap.tile([128, BC], F16, tag=f"wfm_{j}")
                nc.vector.tensor_mul(w, t2_ps[j], q2[j])
                mm = nc.tensor.matmul(aug_ps, onesc, w,
                                      start=(j == 0), stop=(j == KT - 1))
                if j == 0:
                    add_dep_helper(mm.ins, o_mm[KT - 1].ins, sync=False,
                                   reason="pe-order aug after dx")
            final_aug = ap.tile([1, BC], F32, tag="final_aug")
            nc.vector.tensor_copy(final_aug, aug_ps)
            nc.sync.dma_start(out=out_aug[:, :], in_=final_aug)

    nc.finalize()
    return nc


def _get_nc(with_bias23: bool):
    key = bool(with_bias23)
    if key not in _NC:
        _NC[key] = _build(key)
    return _NC[key]


def make_in_maps(inputs):
    f32 = lambda a: np.ascontiguousarray(np.asarray(a), dtype=np.float32)
    f16 = lambda a: np.ascontiguousarray(np.asarray(a, dtype=np.float32),
                                         dtype=np.float16)
    state = f32(inputs["state"])
    t = float(np.asarray(inputs["t"]).reshape(-1)[0])
    W1 = f32(inputs["W1"])
    b1 = f32(inputs["b1"]).reshape(H)
    W2 = f16(inputs["W2"])
    b2 = f32(inputs["b2"]).reshape(H)
    W3 = f16(inputs["W3"])
    b3 = f32(inputs["b3"]).reshape(D)

    with_bias23 = bool(np.any(b2) or np.any(b3))

    b1_eff = b1 + t * W1[D]                  # fold t-row into bias row
    W1a = np.concatenate([W1[:D], b1_eff[None, :]], axis=0)

    W3c = np.concatenate([W3[k * 128:(k + 1) * 128, :] for k in range(KT)],
                         axis=1)
    W1x = W1[:D]
    # W1g[0:64, p*128:(p+1)*128] = W1x cols of m=2p; rows 64:128 m=2p+1
    W1g = np.zeros((128, 2 * 128), np.float32)
    for m in range(KT):
        p, lo = m // 2, 64 * (m % 2)
        W1g[lo:lo + D, p * 128:(p + 1) * 128] = W1x[:, m * 128:(m + 1) * 128]
    base = {
        "W1a": f16(W1a),
        "W2": W2,
        "W3c": np.ascontiguousarray(W3c),
        "W1g": f16(W1g),
        "W3T2": f16(np.concatenate([W3.T, W3.T], axis=0)),
    }
    if with_bias23:
        base["b2r"] = f16(b2.reshape(1, H))
        base["b3r"] = f16(b3.reshape(1, D))
    in_maps = []
    for c in range(NCORES):
        m = dict(base)
        m["stT"] = f16(np.concatenate(
            [state[c * BC:(c + 1) * BC, 1:].T, np.ones((1, BC), np.float32)],
            axis=0))
        in_maps.append(m)
    return with_bias23, in_maps


def kernel(**inputs) -> np.ndarray:
    with_bias23, in_maps = make_in_maps(inputs)
    res = run_bass_kernel_spmd(_get_nc(with_bias23), in_maps,
                               list(range(NCORES))).results
    return np.concatenate(
        [np.concatenate([res[c]["out_aug"].reshape(BC, 1),
                         res[c]["out_dx"]], axis=1)
         for c in range(NCORES)], axis=0)
